# revision 13
# baseline (speedup 1.0000x reference)
"""Bass/Trainium2 kernel for nn_ExpertMLP (soft-blended 8-expert MLP with
BatchNorm between the two layers).

Math (per sample b):
    h  = sum_e coef[b,e] * (x[b] @ w1[e])  + coef[b] @ b1
    hn = (h - mean_B(h)) * rsqrt(var_B(h) + eps) * gamma + beta
    h1 = elu(hn)
    out= sum_e coef[b,e] * (h1[b] @ w2[e]) + coef[b] @ b2

Sharding: HID (512) is split 64-per-core across 8 cores. Each core processes
the FULL batch for its HID slice, so the BatchNorm batch statistics are fully
local (no collective), and the per-expert weights are sharded (not
replicated). Layer 2 contracts only the local HID slice, so each core emits a
partial output [OUT, B]; the host sums the 8 partials, adds the blended-bias
term coef @ (b2 - w2.sum(hid)) (the -w2sum corrects for computing elu+1 on
device), and transposes.

On-chip layout is transposed (features on partitions, batch on the free dim):
  - coef broadcast tiles cbt[p][q*64+r, b] = coef[b, 2p+q] are built on the
    PE: cbt_p = SEL_p^T @ coefT (SEL packed into the cbb constant), copied
    PSUM->SBUF on the (otherwise idle) gpsimd/Pool engine. This replaces 16
    tiny row-DMAs + 4 stream_shuffles (~1.6us of queue time EACH).
  - L1 (k-outer): yp_pair[(e0|e1)*64hid, b] += W1L[p,k].T @ xT[k,b] streams
    matmuls as x chunks land; blend multiplies by cbt on DVE/Pool; a tiled
    identity [I;I|I;I] matmul sums the two expert halves of each pair into
    the h PSUM accumulator (and duplicates h to both partition halves).
  - BN: bn_stats/bn_aggr over the free (batch) dim; rstd = exp(-0.5*ln(v+eps))
    so the ONLY ACT table used the whole kernel is natural_log_exp_and_others
    (Ln/Exp/Relu/Copy) -- no 1.3us mid-kernel table reloads. Dummy warm
    matmuls bridge the serial BN tail so the PE p-state stays at 2.4 GHz,
    and warmup matmuls at t=0 ramp the clock during the initial DMA wait.
  - ELU+1: relu(hn) + min(exp(hn), 1)  (the -1 is folded into the host bias).
  - L2: u_pair = cbt_p (*) [h1;h1]; out_m += W2L[p,m].T @ u_pair.
"""

import sys

sys.path.insert(0, "/opt/trn_rl_repo")

import numpy as np

E, IN, HID, OUT, B = 8, 512, 512, 512, 1024
BN_EPS = 1e-5
N_CORES = 8
HSL = HID // N_CORES  # 64: per-core hid slice
NPAIR = E // 2  # 4 expert pairs
KT1 = IN // 128  # 4 contraction tiles for layer 1
MT2 = OUT // 128  # 4 output row-tiles for layer 2
NBH = B // 512  # 2 batch halves (PSUM free-dim limit)
SELW = NPAIR * 128  # selection-matrix block for the cbt broadcast matmuls
CBB_W = B + 128 + SELW  # packed coefT | b1dup | SEL widths
IN_BF16 = True  # x/w1/w2/cbb in bf16 (halves input DMA; ~0.5% extra err)
OUT_BF16 = True  # partial outputs in bf16 (halves output DMA)
N_WARM = 6  # PE p-state warmup matmuls at t~0
N_FILL = 3  # PE keep-warm matmuls across the serial BN tail

_CACHED = None
_HOST_BIAS_T = None  # [OUT, B] float64, set by make_in_maps


def build_nc(n_reps: int = 1, trace_sim: bool = False, serialize_reps: bool = True,
             loop_iters: int = 0):
    """Build + compile the (SPMD, identical-program) Bass module."""
    from contextlib import ExitStack, nullcontext

    import concourse.bass as bass
    import concourse.tile as tile
    from concourse import bacc, mybir

    f32 = mybir.dt.float32
    f32r = mybir.dt.float32r
    bf16 = mybir.dt.bfloat16
    mdt = bf16 if IN_BF16 else f32r
    odt = bf16 if OUT_BF16 else f32
    Alu = mybir.AluOpType
    Act = mybir.ActivationFunctionType

    nc = bacc.Bacc(
        "TRN2", target_bir_lowering=False, debug=False, num_devices=N_CORES
    )

    xT = nc.dram_tensor("xT", [128, KT1, B], mdt, kind="ExternalInput")
    w1l = nc.dram_tensor("w1l", [128, NPAIR * KT1 * 128], mdt, kind="ExternalInput")
    w2l = nc.dram_tensor("w2l", [128, NPAIR * MT2 * 128], mdt, kind="ExternalInput")
    cbb = nc.dram_tensor("cbb", [E, CBB_W], mdt, kind="ExternalInput")
    gb = nc.dram_tensor("gb", [128, 2], f32, kind="ExternalInput")
    idd = nc.dram_tensor("idd", [128, 128], f32r, kind="ExternalInput")
    outT = nc.dram_tensor("outT", [OUT, B], odt, kind="ExternalOutput")

    with ExitStack() as ctx:
        tc = ctx.enter_context(tile.TileContext(nc, trace_sim=trace_sim))
        ins = ctx.enter_context(tc.tile_pool(name="ins", bufs=2))
        cbts = ctx.enter_context(tc.tile_pool(name="cbts", bufs=6))
        blend = ctx.enter_context(tc.tile_pool(name="blend", bufs=3))
        mids = ctx.enter_context(tc.tile_pool(name="mids", bufs=2))
        small = ctx.enter_context(tc.tile_pool(name="small", bufs=2))
        ups = ctx.enter_context(tc.tile_pool(name="ups", bufs=6))
        outs = ctx.enter_context(tc.tile_pool(name="outs", bufs=4))
        yps = ctx.enter_context(tc.tile_pool(name="yps", bufs=4, space="PSUM"))
        hps = ctx.enter_context(tc.tile_pool(name="hps", bufs=2, space="PSUM"))
        ops = ctx.enter_context(tc.tile_pool(name="ops", bufs=2, space="PSUM"))

        loop_cm = tc.For_i(0, loop_iters, 1) if loop_iters else nullcontext()
        with loop_cm:
          for _rep in range(n_reps):
            if _rep > 0 and serialize_reps:
                with tc.tile_critical():
                    nc.all_engine_barrier()

            # ---- t~0: scratch init + ACT table warm (Ln/Exp set) ------------
            warm = ins.tile([128, 640], mdt, tag="warm", bufs=2)
            if mdt == f32r:
                nc.vector.memset(warm[:].bitcast(f32), 0.0)
            else:
                nc.vector.memset(warm[:].bitcast(mybir.dt.uint16), 0)
            eps = small.tile([128, 1], f32, tag="eps")
            nc.vector.memset(eps, BN_EPS)
            tw = small.tile([128, 1], f32, tag="tw")
            nc.scalar.activation(tw[:], eps[:], Act.Ln)
            nc.scalar.activation(tw[:], tw[:], Act.Exp)

            # ---- input loads: spread across SP/ACT HWDGE + DVE HWDGE +
            # Pool SWDGE (each dma_start holds its queue for the whole
            # transfer in the HW model, so few-but-wide DMAs on many queues).
            # Need-order: {x-h0, w1, cbb} gate layer 1, so each rides its
            # own queue; second wave brings x-h1 / w2 / idd / gb.
            xall = ins.tile([128, KT1, B], mdt, tag="xts", name="xts", bufs=2)
            w1all = ins.tile([128, NPAIR, KT1, 128], mdt, tag="w1t",
                             name="w1t", bufs=2)
            w2all = ins.tile([128, NPAIR, MT2, 128], mdt, tag="w2t",
                             name="w2t", bufs=2)
            cbbt = ins.tile([E, CBB_W], mdt, tag="cbbt")
            iddt = ins.tile([128, 128], f32r, tag="iddt")
            gbt = small.tile([128, 2], f32, tag="gbt")
            w1v = w1all.rearrange("i p k j -> i (p k j)")
            w2v = w2all.rearrange("i p m j -> i (p m j)")
            nc.sync.dma_start(xall[:, :, 0:512], xT[:, :, 0:512])
            nc.scalar.dma_start(w1v[:], w1l[:])
            nc.gpsimd.dma_start(cbbt[:], cbb[:])
            nc.gpsimd.dma_start(xall[:, :, 512:1024], xT[:, :, 512:1024])
            nc.scalar.dma_start(w2v[:], w2l[:])
            nc.sync.dma_start(iddt[:], idd[:])
            nc.sync.dma_start(gbt[:], gb[:])

            ct = cbbt[:, 0:B]
            b1t = cbbt[:, B : B + 128]
            selt = cbbt[:, B + 128 : CBB_W]
            gm = gbt[:, 0:1]
            bt = gbt[:, 1:2]

            # ---- PE p-state ramp during the DMA wait ------------------------
            for _w in range(N_WARM):
                wps = ops.tile([128, 512], f32, tag="ops", name="wps")
                nc.tensor.matmul(
                    wps[:], warm[:, 512:640], warm[:, 0:512],
                    start=True, stop=True,
                )

            # ---- coef broadcast tiles via PE + Pool copies ------------------
            ctiles = []
            for p in range(NPAIR):
                t = cbts.tile([128, B], mdt, tag="cbt", name="cbt")
                ctiles.append(t)
                for bh in range(NBH):
                    bsl = slice(bh * 512, (bh + 1) * 512)
                    cp = ops.tile([128, 512], f32, tag="ops", name="cp")
                    nc.tensor.matmul(
                        cp[:], selt[:, p * 128 : (p + 1) * 128], ct[:, bsl],
                        start=True, stop=True,
                    )
                    # gpsimd cannot read PSUM; split drains across DVE/ACT
                    if (p * NBH + bh) % 2 == 0:
                        nc.vector.tensor_copy(t[:, bsl], cp[:])
                    else:
                        nc.scalar.copy(t[:, bsl], cp[:])
            cbt = ctiles

            # ---- layer 1 (k-outer) + blend + pair-sum -----------------------
            h_ps = []
            stats = small.tile([128, NBH, 6], f32, tag="stats")
            for bh in range(NBH):
                bsl = slice(bh * 512, (bh + 1) * 512)
                hp = hps.tile([128, 512], f32, tag="hps")
                nc.tensor.matmul(hp[:], b1t, ct[:, bsl], start=True, stop=False)
                yts = []
                for k in range(KT1):
                    for p in range(NPAIR):
                        if k == 0:
                            yts.append(yps.tile([128, 512], f32, tag="yps",
                                                name="yp"))
                        nc.tensor.matmul(
                            yts[p][:], w1all[:, p, k, :], xall[:, k, bsl],
                            start=(k == 0), stop=(k == KT1 - 1),
                        )
                for p in range(NPAIR):
                    bl = blend.tile([128, 512], f32r, tag="bl")
                    nc.vector.tensor_mul(bl[:], yts[p][:], cbt[p][:, bsl])
                    nc.tensor.matmul(
                        hp[:], iddt[:], bl[:], start=False, stop=(p == NPAIR - 1)
                    )
                nc.vector.bn_stats(out=stats[:, bh, :], in_=hp[:])
                h_ps.append(hp)

            # ---- batch-norm scale/bias --------------------------------------
            # rstd = exp(-0.5*ln(var+eps)): stays on the Ln/Exp ACT table.
            # (dummy matmuls keep the PE p-state hot through the serial tail)
            mv = small.tile([128, 2], f32, tag="mv")
            nc.vector.bn_aggr(out=mv[:], in_=stats[:])
            for _w in range(N_FILL):
                wps = ops.tile([128, 512], f32, tag="ops", name="wps")
                nc.tensor.matmul(
                    wps[:], warm[:, 512:640], warm[:, 0:512],
                    start=True, stop=True,
                )
            lnv = small.tile([128, 1], f32, tag="lnv")
            nc.scalar.activation(lnv[:], mv[:, 1:2], Act.Ln, bias=eps[:])
            rstd = small.tile([128, 1], f32, tag="rstd")
            nc.scalar.activation(rstd[:], lnv[:], Act.Exp, scale=-0.5)
            ns = small.tile([128, 1], f32, tag="ns")
            nc.vector.tensor_mul(ns[:], rstd[:], gm)
            nb0 = small.tile([128, 1], f32, tag="nb0")
            nc.vector.tensor_mul(nb0[:], mv[:, 0:1], ns[:])
            nb = small.tile([128, 1], f32, tag="nb")
            nc.vector.tensor_sub(nb[:], bt, nb0[:])

            # ---- ELU+1 + blend + layer 2 ------------------------------------
            oi = 0
            for bh in range(NBH):
                bsl = slice(bh * 512, (bh + 1) * 512)
                expd = mids.tile([128, 512], f32, tag="expd")
                nc.scalar.activation(
                    expd[:], h_ps[bh][:], Act.Exp, bias=nb[:], scale=ns[:]
                )
                rl = mids.tile([128, 512], f32, tag="rl")
                nc.scalar.activation(
                    rl[:], h_ps[bh][:], Act.Relu, bias=nb[:], scale=ns[:]
                )
                h1 = mids.tile([128, 512], mdt, tag="h1")
                nc.vector.scalar_tensor_tensor(
                    out=h1[:], in0=expd[:], scalar=1.0, in1=rl[:],
                    op0=Alu.min, op1=Alu.add,
                )
                us = []
                for p in range(NPAIR):
                    u = ups.tile([128, 512], mdt, tag="u", name="u")
                    ueng = nc.vector if p % 2 == 0 else nc.gpsimd
                    ueng.tensor_mul(u[:], cbt[p][:, bsl], h1[:])
                    us.append(u)
                for m in range(MT2):
                    op = ops.tile([128, 512], f32, tag="ops")
                    for p in range(NPAIR):
                        nc.tensor.matmul(
                            op[:], w2all[:, p, m, :], us[p][:],
                            start=(p == 0), stop=(p == NPAIR - 1),
                        )
                    ot = outs.tile([128, 512], odt, tag="ot", name="ot")
                    if oi % 2 == 0:
                        nc.vector.tensor_copy(ot[:], op[:])
                    else:
                        nc.scalar.copy(ot[:], op[:])
                    seng = [nc.gpsimd, nc.sync, nc.scalar, nc.gpsimd,
                            nc.gpsimd, nc.sync, nc.scalar, nc.sync][oi % 8]
                    seng.dma_start(outT[m * 128 : (m + 1) * 128, bsl], ot[:])
                    oi += 1

    nc.compile()
    return nc


def make_in_maps(x, blending_coef, w1, b1, w2, b2, gamma, beta):
    """Host-side input marshaling: per-core weight slices + shared tensors."""
    global _HOST_BIAS_T
    import ml_dtypes

    f32 = np.float32
    mmdt = ml_dtypes.bfloat16 if IN_BF16 else f32
    x = np.asarray(x, f32)
    coef = np.asarray(blending_coef, f32)
    w1 = np.asarray(w1, f32)
    b1 = np.asarray(b1, f32)
    w2 = np.asarray(w2, f32)
    b2 = np.asarray(b2, f32)
    gamma = np.asarray(gamma, f32)
    beta = np.asarray(beta, f32)

    # blended bias for layer 2, including the correction for computing
    # (elu+1) on device:  out_true = out_dev + coef @ (b2 - sum_h w2[:,h,:])
    _HOST_BIAS_T = (
        coef.astype(np.float64) @ (b2 - w2.sum(axis=1)).astype(np.float64)
    ).T

    xT = np.ascontiguousarray(
        x.T.reshape(KT1, 128, B).transpose(1, 0, 2)
    )  # [128, KT1, B]
    coefT = np.ascontiguousarray(coef.T)
    idd = np.ascontiguousarray(np.tile(np.eye(64, dtype=f32), (2, 2)))
    sel = np.zeros((E, SELW), f32)
    for p in range(NPAIR):
        for q in range(2):
            sel[2 * p + q, p * 128 + q * 64 : p * 128 + (q + 1) * 64] = 1.0

    in_maps = []
    for c in range(N_CORES):
        sl = slice(c * HSL, (c + 1) * HSL)
        w1s = w1[:, :, sl]  # [E, IN, 64]
        w1L = np.ascontiguousarray(
            w1s.reshape(NPAIR, 2, KT1, 128, HSL)
            .transpose(3, 0, 2, 1, 4)  # [i, pair, k, eo, j]
            .reshape(128, NPAIR * KT1 * 128)
        )
        w2s = w2[:, sl, :]  # [E, 64, OUT]
        w2L = np.ascontiguousarray(
            w2s.reshape(NPAIR, 2, HSL, MT2, 128)
            .transpose(1, 2, 0, 3, 4)  # [eo, h, pair, m, j]
            .reshape(128, NPAIR * MT2 * 128)
        )
        cbbv = np.concatenate([coefT, np.tile(b1[:, sl], (1, 2)), sel], axis=1)
        gbv = np.stack(
            [np.tile(gamma[sl], 2), np.tile(beta[sl], 2)], axis=1
        )
        in_maps.append(
            {
                "xT": xT.astype(mmdt),
                "w1l": w1L.astype(mmdt),
                "w2l": w2L.astype(mmdt),
                "cbb": np.ascontiguousarray(cbbv).astype(mmdt),
                "gb": np.ascontiguousarray(gbv),
                "idd": idd,
            }
        )
    return in_maps


def combine_outputs(per_core_outs):
    """Sum per-core partial [OUT, B] outputs + host bias; return [B, OUT]."""
    acc = np.zeros((OUT, B), np.float64)
    for o in per_core_outs:
        acc += np.asarray(o, np.float64)
    if _HOST_BIAS_T is not None:
        acc += _HOST_BIAS_T
    return np.ascontiguousarray(acc.T.astype(np.float32))


def kernel(x, blending_coef, w1, b1, w2, b2, gamma, beta):
    global _CACHED
    from concourse.bass_utils import run_bass_kernel_spmd

    if _CACHED is None:
        _CACHED = build_nc(n_reps=1)
    nc = _CACHED
    in_maps = make_in_maps(x, blending_coef, w1, b1, w2, b2, gamma, beta)
    res = run_bass_kernel_spmd(nc, in_maps, list(range(N_CORES)))
    return combine_outputs([res.results[c]["outT"] for c in range(N_CORES)])


# revision 14
# speedup vs baseline: 1.1203x; 1.1203x over previous
"""Bass/Trainium2 kernel for nn_ExpertMLP (soft-blended 8-expert MLP with
BatchNorm between the two layers).

Math (per sample b):
    h  = sum_e coef[b,e] * (x[b] @ w1[e])  + coef[b] @ b1
    hn = (h - mean_B(h)) * rsqrt(var_B(h) + eps) * gamma + beta
    h1 = elu(hn)
    out= sum_e coef[b,e] * (h1[b] @ w2[e]) + coef[b] @ b2

Sharding: HID (512) is split 64-per-core across 8 cores. Each core processes
the FULL batch for its HID slice, so the BatchNorm batch statistics are fully
local (no collective), and the per-expert weights are sharded (not
replicated). Layer 2 contracts only the local HID slice, so each core emits a
partial output [OUT, B]; the host sums the 8 partials, adds the blended-bias
term coef @ (b2 - w2.sum(hid)) (the -w2sum corrects for computing elu+1 on
device), and transposes.

On-chip layout is transposed (features on partitions, batch on the free dim):
  - coef broadcast tiles cbt[p][q*64+r, b] = coef[b, 2p+q] are built on the
    PE: cbt_p = SEL_p^T @ coefT (SEL packed into the cbb constant), copied
    PSUM->SBUF on the (otherwise idle) gpsimd/Pool engine. This replaces 16
    tiny row-DMAs + 4 stream_shuffles (~1.6us of queue time EACH).
  - L1 (k-outer): yp_pair[(e0|e1)*64hid, b] += W1L[p,k].T @ xT[k,b] streams
    matmuls as x chunks land; blend multiplies by cbt on DVE/Pool; a tiled
    identity [I;I|I;I] matmul sums the two expert halves of each pair into
    the h PSUM accumulator (and duplicates h to both partition halves).
  - BN: bn_stats/bn_aggr over the free (batch) dim; rstd = exp(-0.5*ln(v+eps))
    so the ONLY ACT table used the whole kernel is natural_log_exp_and_others
    (Ln/Exp/Relu/Copy) -- no 1.3us mid-kernel table reloads. Dummy warm
    matmuls bridge the serial BN tail so the PE p-state stays at 2.4 GHz,
    and warmup matmuls at t=0 ramp the clock during the initial DMA wait.
  - ELU+1: relu(hn) + min(exp(hn), 1)  (the -1 is folded into the host bias).
  - L2: u_pair = cbt_p (*) [h1;h1]; out_m += W2L[p,m].T @ u_pair.
"""

import sys

sys.path.insert(0, "/opt/trn_rl_repo")

import numpy as np

E, IN, HID, OUT, B = 8, 512, 512, 512, 1024
BN_EPS = 1e-5
N_CORES = 8
HSL = HID // N_CORES  # 64: per-core hid slice
NPAIR = E // 2  # 4 expert pairs
KT1 = IN // 128  # 4 contraction tiles for layer 1
MT2 = OUT // 128  # 4 output row-tiles for layer 2
NBH = B // 512  # 2 batch halves (PSUM free-dim limit)
SELW = NPAIR * 128  # selection-matrix block for the cbt broadcast matmuls
CBB_W = B + 128 + SELW  # packed coefT | b1dup | SEL widths
IN_BF16 = True  # x/w1/w2/cbb in bf16 (halves input DMA; ~0.5% extra err)
OUT_BF16 = True  # partial outputs in bf16 (halves output DMA)
N_WARM = 6  # PE p-state warmup matmuls at t~0
N_FILL = 3  # PE keep-warm matmuls across the serial BN tail

_CACHED = None
_HOST_BIAS_T = None  # [OUT, B] float64, set by make_in_maps


def build_nc(n_reps: int = 1, trace_sim: bool = False, serialize_reps: bool = True,
             loop_iters: int = 0):
    """Build + compile the (SPMD, identical-program) Bass module."""
    from contextlib import ExitStack, nullcontext

    import concourse.bass as bass
    import concourse.tile as tile
    from concourse import bacc, mybir

    f32 = mybir.dt.float32
    f32r = mybir.dt.float32r
    bf16 = mybir.dt.bfloat16
    mdt = bf16 if IN_BF16 else f32r
    odt = bf16 if OUT_BF16 else f32
    Alu = mybir.AluOpType
    Act = mybir.ActivationFunctionType

    nc = bacc.Bacc(
        "TRN2", target_bir_lowering=False, debug=False, num_devices=N_CORES
    )

    xT = nc.dram_tensor("xT", [128, KT1, B], mdt, kind="ExternalInput")
    w1l = nc.dram_tensor("w1l", [128, NPAIR * KT1 * 128], mdt, kind="ExternalInput")
    w2l = nc.dram_tensor("w2l", [128, NPAIR * MT2 * 128], mdt, kind="ExternalInput")
    cbb = nc.dram_tensor("cbb", [E, CBB_W], mdt, kind="ExternalInput")
    gb = nc.dram_tensor("gb", [128, 2], f32, kind="ExternalInput")
    idd = nc.dram_tensor("idd", [128, 128], f32r, kind="ExternalInput")
    outT = nc.dram_tensor("outT", [OUT, B], odt, kind="ExternalOutput")

    with ExitStack() as ctx:
        tc = ctx.enter_context(tile.TileContext(nc, trace_sim=trace_sim))
        ins = ctx.enter_context(tc.tile_pool(name="ins", bufs=2))
        cbts = ctx.enter_context(tc.tile_pool(name="cbts", bufs=6))
        blend = ctx.enter_context(tc.tile_pool(name="blend", bufs=3))
        mids = ctx.enter_context(tc.tile_pool(name="mids", bufs=2))
        small = ctx.enter_context(tc.tile_pool(name="small", bufs=2))
        ups = ctx.enter_context(tc.tile_pool(name="ups", bufs=6))
        outs = ctx.enter_context(tc.tile_pool(name="outs", bufs=4))
        yps = ctx.enter_context(tc.tile_pool(name="yps", bufs=4, space="PSUM"))
        hps = ctx.enter_context(tc.tile_pool(name="hps", bufs=2, space="PSUM"))
        ops = ctx.enter_context(tc.tile_pool(name="ops", bufs=2, space="PSUM"))

        loop_cm = tc.For_i(0, loop_iters, 1) if loop_iters else nullcontext()
        with loop_cm:
          for _rep in range(n_reps):
            if _rep > 0 and serialize_reps:
                with tc.tile_critical():
                    nc.all_engine_barrier()

            # ---- t~0: scratch init + ACT table warm (Ln/Exp set) ------------
            warm = ins.tile([128, 640], mdt, tag="warm", bufs=2)
            if mdt == f32r:
                nc.vector.memset(warm[:].bitcast(f32), 0.0)
            else:
                nc.vector.memset(warm[:].bitcast(mybir.dt.uint16), 0)
            eps = small.tile([128, 1], f32, tag="eps")
            nc.vector.memset(eps, BN_EPS)
            tw = small.tile([128, 1], f32, tag="tw")
            nc.scalar.activation(tw[:], eps[:], Act.Ln)
            nc.scalar.activation(tw[:], tw[:], Act.Exp)

            # ---- input loads: spread across SP/ACT HWDGE + DVE HWDGE +
            # Pool SWDGE (each dma_start holds its queue for the whole
            # transfer in the HW model, so few-but-wide DMAs on many queues).
            # Need-order: {x-h0, w1, cbb} gate layer 1, so each rides its
            # own queue; second wave brings x-h1 / w2 / idd / gb.
            xall = ins.tile([128, KT1, B], mdt, tag="xts", name="xts", bufs=2)
            w1all = ins.tile([128, NPAIR, KT1, 128], mdt, tag="w1t",
                             name="w1t", bufs=2)
            w2all = ins.tile([128, NPAIR, MT2, 128], mdt, tag="w2t",
                             name="w2t", bufs=2)
            cbbt = ins.tile([E, CBB_W], mdt, tag="cbbt")
            iddt = ins.tile([128, 128], f32r, tag="iddt")
            gbt = small.tile([128, 2], f32, tag="gbt")
            w1v = w1all.rearrange("i p k j -> i (p k j)")
            w2v = w2all.rearrange("i p m j -> i (p m j)")
            nc.scalar.dma_start(cbbt[:], cbb[:])
            nc.sync.dma_start(w1v[:], w1l[:])
            for k in range(KT1):
                qs = nc.sync if k % 2 == 0 else nc.scalar
                qs.dma_start(xall[:, k, 0:512], xT[:, k, 0:512])
            nc.scalar.dma_start(iddt[:], idd[:])
            nc.sync.dma_start(gbt[:], gb[:])
            for k in range(KT1):
                qs = nc.sync if k % 2 == 0 else nc.scalar
                qs.dma_start(xall[:, k, 512:1024], xT[:, k, 512:1024])
            nc.sync.dma_start(w2v[:], w2l[:])

            ct = cbbt[:, 0:B]
            b1t = cbbt[:, B : B + 128]
            selt = cbbt[:, B + 128 : CBB_W]
            gm = gbt[:, 0:1]
            bt = gbt[:, 1:2]

            # ---- PE p-state ramp during the DMA wait ------------------------
            for _w in range(N_WARM):
                wps = ops.tile([128, 512], f32, tag="ops", name="wps")
                nc.tensor.matmul(
                    wps[:], warm[:, 512:640], warm[:, 0:512],
                    start=True, stop=True,
                )

            # ---- coef broadcast tiles via PE + Pool copies ------------------
            ctiles = []
            for p in range(NPAIR):
                t = cbts.tile([128, B], mdt, tag="cbt", name="cbt")
                ctiles.append(t)
                for bh in range(NBH):
                    bsl = slice(bh * 512, (bh + 1) * 512)
                    cp = ops.tile([128, 512], f32, tag="ops", name="cp")
                    nc.tensor.matmul(
                        cp[:], selt[:, p * 128 : (p + 1) * 128], ct[:, bsl],
                        start=True, stop=True,
                    )
                    # gpsimd cannot read PSUM; split drains across DVE/ACT
                    if (p * NBH + bh) % 2 == 0:
                        nc.vector.tensor_copy(t[:, bsl], cp[:])
                    else:
                        nc.scalar.copy(t[:, bsl], cp[:])
            cbt = ctiles

            # ---- layer 1 (k-outer) + blend + pair-sum -----------------------
            h_ps = []
            stats = small.tile([128, NBH, 6], f32, tag="stats")
            for bh in range(NBH):
                bsl = slice(bh * 512, (bh + 1) * 512)
                hp = hps.tile([128, 512], f32, tag="hps")
                nc.tensor.matmul(hp[:], b1t, ct[:, bsl], start=True, stop=False)
                yts = []
                for k in range(KT1):
                    for p in range(NPAIR):
                        if k == 0:
                            yts.append(yps.tile([128, 512], f32, tag="yps",
                                                name="yp"))
                        nc.tensor.matmul(
                            yts[p][:], w1all[:, p, k, :], xall[:, k, bsl],
                            start=(k == 0), stop=(k == KT1 - 1),
                        )
                for p in range(NPAIR):
                    bl = blend.tile([128, 512], f32r, tag="bl")
                    nc.vector.tensor_mul(bl[:], yts[p][:], cbt[p][:, bsl])
                    nc.tensor.matmul(
                        hp[:], iddt[:], bl[:], start=False, stop=(p == NPAIR - 1)
                    )
                nc.vector.bn_stats(out=stats[:, bh, :], in_=hp[:])
                h_ps.append(hp)

            # ---- batch-norm scale/bias --------------------------------------
            # rstd = exp(-0.5*ln(var+eps)): stays on the Ln/Exp ACT table.
            # (dummy matmuls keep the PE p-state hot through the serial tail)
            mv = small.tile([128, 2], f32, tag="mv")
            nc.vector.bn_aggr(out=mv[:], in_=stats[:])
            for _w in range(N_FILL):
                wps = ops.tile([128, 512], f32, tag="ops", name="wps")
                nc.tensor.matmul(
                    wps[:], warm[:, 512:640], warm[:, 0:512],
                    start=True, stop=True,
                )
            lnv = small.tile([128, 1], f32, tag="lnv")
            nc.scalar.activation(lnv[:], mv[:, 1:2], Act.Ln, bias=eps[:])
            rstd = small.tile([128, 1], f32, tag="rstd")
            nc.scalar.activation(rstd[:], lnv[:], Act.Exp, scale=-0.5)
            ns = small.tile([128, 1], f32, tag="ns")
            nc.vector.tensor_mul(ns[:], rstd[:], gm)
            nb0 = small.tile([128, 1], f32, tag="nb0")
            nc.vector.tensor_mul(nb0[:], mv[:, 0:1], ns[:])
            nb = small.tile([128, 1], f32, tag="nb")
            nc.vector.tensor_sub(nb[:], bt, nb0[:])

            # ---- ELU+1 + blend + layer 2 ------------------------------------
            oi = 0
            for bh in range(NBH):
                bsl = slice(bh * 512, (bh + 1) * 512)
                expd = mids.tile([128, 512], f32, tag="expd")
                nc.scalar.activation(
                    expd[:], h_ps[bh][:], Act.Exp, bias=nb[:], scale=ns[:]
                )
                rl = mids.tile([128, 512], f32, tag="rl")
                nc.scalar.activation(
                    rl[:], h_ps[bh][:], Act.Relu, bias=nb[:], scale=ns[:]
                )
                h1 = mids.tile([128, 512], mdt, tag="h1")
                nc.vector.scalar_tensor_tensor(
                    out=h1[:], in0=expd[:], scalar=1.0, in1=rl[:],
                    op0=Alu.min, op1=Alu.add,
                )
                us = []
                for p in range(NPAIR):
                    u = ups.tile([128, 512], mdt, tag="u", name="u")
                    ueng = nc.vector if p % 2 == 0 else nc.gpsimd
                    ueng.tensor_mul(u[:], cbt[p][:, bsl], h1[:])
                    us.append(u)
                for m in range(MT2):
                    op = ops.tile([128, 512], f32, tag="ops")
                    for p in range(NPAIR):
                        nc.tensor.matmul(
                            op[:], w2all[:, p, m, :], us[p][:],
                            start=(p == 0), stop=(p == NPAIR - 1),
                        )
                    ot = outs.tile([128, 512], odt, tag="ot", name="ot")
                    if oi % 2 == 0:
                        nc.vector.tensor_copy(ot[:], op[:])
                    else:
                        nc.scalar.copy(ot[:], op[:])
                    seng = [nc.gpsimd, nc.sync, nc.scalar, nc.gpsimd,
                            nc.gpsimd, nc.sync, nc.scalar, nc.sync][oi % 8]
                    seng.dma_start(outT[m * 128 : (m + 1) * 128, bsl], ot[:])
                    oi += 1

    nc.compile()
    return nc


def make_in_maps(x, blending_coef, w1, b1, w2, b2, gamma, beta):
    """Host-side input marshaling: per-core weight slices + shared tensors."""
    global _HOST_BIAS_T
    import ml_dtypes

    f32 = np.float32
    mmdt = ml_dtypes.bfloat16 if IN_BF16 else f32
    x = np.asarray(x, f32)
    coef = np.asarray(blending_coef, f32)
    w1 = np.asarray(w1, f32)
    b1 = np.asarray(b1, f32)
    w2 = np.asarray(w2, f32)
    b2 = np.asarray(b2, f32)
    gamma = np.asarray(gamma, f32)
    beta = np.asarray(beta, f32)

    # blended bias for layer 2, including the correction for computing
    # (elu+1) on device:  out_true = out_dev + coef @ (b2 - sum_h w2[:,h,:])
    _HOST_BIAS_T = (
        coef.astype(np.float64) @ (b2 - w2.sum(axis=1)).astype(np.float64)
    ).T

    xT = np.ascontiguousarray(
        x.T.reshape(KT1, 128, B).transpose(1, 0, 2)
    )  # [128, KT1, B]
    coefT = np.ascontiguousarray(coef.T)
    idd = np.ascontiguousarray(np.tile(np.eye(64, dtype=f32), (2, 2)))
    sel = np.zeros((E, SELW), f32)
    for p in range(NPAIR):
        for q in range(2):
            sel[2 * p + q, p * 128 + q * 64 : p * 128 + (q + 1) * 64] = 1.0

    in_maps = []
    for c in range(N_CORES):
        sl = slice(c * HSL, (c + 1) * HSL)
        w1s = w1[:, :, sl]  # [E, IN, 64]
        w1L = np.ascontiguousarray(
            w1s.reshape(NPAIR, 2, KT1, 128, HSL)
            .transpose(3, 0, 2, 1, 4)  # [i, pair, k, eo, j]
            .reshape(128, NPAIR * KT1 * 128)
        )
        w2s = w2[:, sl, :]  # [E, 64, OUT]
        w2L = np.ascontiguousarray(
            w2s.reshape(NPAIR, 2, HSL, MT2, 128)
            .transpose(1, 2, 0, 3, 4)  # [eo, h, pair, m, j]
            .reshape(128, NPAIR * MT2 * 128)
        )
        cbbv = np.concatenate([coefT, np.tile(b1[:, sl], (1, 2)), sel], axis=1)
        gbv = np.stack(
            [np.tile(gamma[sl], 2), np.tile(beta[sl], 2)], axis=1
        )
        in_maps.append(
            {
                "xT": xT.astype(mmdt),
                "w1l": w1L.astype(mmdt),
                "w2l": w2L.astype(mmdt),
                "cbb": np.ascontiguousarray(cbbv).astype(mmdt),
                "gb": np.ascontiguousarray(gbv),
                "idd": idd,
            }
        )
    return in_maps


def combine_outputs(per_core_outs):
    """Sum per-core partial [OUT, B] outputs + host bias; return [B, OUT]."""
    acc = np.zeros((OUT, B), np.float64)
    for o in per_core_outs:
        acc += np.asarray(o, np.float64)
    if _HOST_BIAS_T is not None:
        acc += _HOST_BIAS_T
    return np.ascontiguousarray(acc.T.astype(np.float32))


def kernel(x, blending_coef, w1, b1, w2, b2, gamma, beta):
    global _CACHED
    from concourse.bass_utils import run_bass_kernel_spmd

    if _CACHED is None:
        _CACHED = build_nc(n_reps=1)
    nc = _CACHED
    in_maps = make_in_maps(x, blending_coef, w1, b1, w2, b2, gamma, beta)
    res = run_bass_kernel_spmd(nc, in_maps, list(range(N_CORES)))
    return combine_outputs([res.results[c]["outT"] for c in range(N_CORES)])


# revision 18
# speedup vs baseline: 1.2114x; 1.0814x over previous
"""Bass/Trainium2 kernel for nn_ExpertMLP (soft-blended 8-expert MLP with
BatchNorm between the two layers).

Math (per sample b):
    h  = sum_e coef[b,e] * (x[b] @ w1[e])  + coef[b] @ b1
    hn = (h - mean_B(h)) * rsqrt(var_B(h) + eps) * gamma + beta
    h1 = elu(hn)
    out= sum_e coef[b,e] * (h1[b] @ w2[e]) + coef[b] @ b2

Sharding: HID (512) is split 64-per-core across 8 cores. Each core processes
the FULL batch for its HID slice, so the BatchNorm batch statistics are fully
local (no collective), and the per-expert weights are sharded (not
replicated). Layer 2 contracts only the local HID slice, so each core emits a
partial output [OUT, B]; the host sums the 8 partials, adds the blended-bias
term coef @ (b2 - w2.sum(hid)) (the -w2sum corrects for computing elu+1 on
device), and transposes.

On-chip layout is transposed (features on partitions, batch on the free dim):
  - coef broadcast tiles cbt[p][q*64+r, b] = coef[b, 2p+q] are built on the
    PE: cbt_p = SEL_p^T @ coefT (SEL packed into the cbb constant), copied
    PSUM->SBUF on the (otherwise idle) gpsimd/Pool engine. This replaces 16
    tiny row-DMAs + 4 stream_shuffles (~1.6us of queue time EACH).
  - L1 (k-outer): yp_pair[(e0|e1)*64hid, b] += W1L[p,k].T @ xT[k,b] streams
    matmuls as x chunks land; blend multiplies by cbt on DVE/Pool; a tiled
    identity [I;I|I;I] matmul sums the two expert halves of each pair into
    the h PSUM accumulator (and duplicates h to both partition halves).
  - BN: bn_stats/bn_aggr over the free (batch) dim; rstd = exp(-0.5*ln(v+eps))
    so the ONLY ACT table used the whole kernel is natural_log_exp_and_others
    (Ln/Exp/Relu/Copy) -- no 1.3us mid-kernel table reloads. Dummy warm
    matmuls bridge the serial BN tail so the PE p-state stays at 2.4 GHz,
    and warmup matmuls at t=0 ramp the clock during the initial DMA wait.
  - ELU+1: relu(hn) + min(exp(hn), 1)  (the -1 is folded into the host bias).
  - L2: u_pair = cbt_p (*) [h1;h1]; out_m += W2L[p,m].T @ u_pair.
"""

import sys

sys.path.insert(0, "/opt/trn_rl_repo")

import numpy as np

E, IN, HID, OUT, B = 8, 512, 512, 512, 1024
BN_EPS = 1e-5
N_CORES = 8
HSL = HID // N_CORES  # 64: per-core hid slice
NPAIR = E // 2  # 4 expert pairs
KT1 = IN // 128  # 4 contraction tiles for layer 1
MT2 = OUT // 128  # 4 output row-tiles for layer 2
NBH = B // 512  # 2 batch halves (PSUM free-dim limit)
SELW = NPAIR * 128  # selection-matrix block for the cbt broadcast matmuls
CBB_W = B + 128 + SELW  # packed coefT | b1dup | SEL widths
IN_BF16 = True  # x/w1/w2/cbb in bf16 (halves input DMA; ~0.5% extra err)
OUT_BF16 = True  # partial outputs in bf16 (halves output DMA)
N_WARM = 6  # PE p-state warmup matmuls at t~0
N_FILL = 3  # PE keep-warm matmuls across the serial BN tail

_CACHED = None
_HOST_BIAS_T = None  # [OUT, B] float64, set by make_in_maps


def build_nc(n_reps: int = 1, trace_sim: bool = False, serialize_reps: bool = True,
             loop_iters: int = 0, phase: int = 3):
    """Build + compile the (SPMD, identical-program) Bass module."""
    from contextlib import ExitStack, nullcontext

    import concourse.bass as bass
    import concourse.tile as tile
    from concourse import bacc, mybir

    f32 = mybir.dt.float32
    f32r = mybir.dt.float32r
    bf16 = mybir.dt.bfloat16
    mdt = bf16 if IN_BF16 else f32r
    odt = bf16 if OUT_BF16 else f32
    Alu = mybir.AluOpType
    Act = mybir.ActivationFunctionType

    nc = bacc.Bacc(
        "TRN2", target_bir_lowering=False, debug=False, num_devices=N_CORES
    )

    xT = nc.dram_tensor("xT", [128, KT1, B], mdt, kind="ExternalInput")
    w1l = nc.dram_tensor("w1l", [128, NPAIR * KT1 * 128], mdt, kind="ExternalInput")
    w2l = nc.dram_tensor("w2l", [128, NPAIR * MT2 * 128], mdt, kind="ExternalInput")
    cbb = nc.dram_tensor("cbb", [E, CBB_W], mdt, kind="ExternalInput")
    gb = nc.dram_tensor("gb", [128, 2], f32, kind="ExternalInput")
    idd = nc.dram_tensor("idd", [128, 128], f32r, kind="ExternalInput")
    outT = nc.dram_tensor("outT", [OUT, B], odt, kind="ExternalOutput")

    with ExitStack() as ctx:
        tc = ctx.enter_context(tile.TileContext(nc, trace_sim=trace_sim))
        ins = ctx.enter_context(tc.tile_pool(name="ins", bufs=2))
        cbts = ctx.enter_context(tc.tile_pool(name="cbts", bufs=6))
        blend = ctx.enter_context(tc.tile_pool(name="blend", bufs=3))
        mids = ctx.enter_context(tc.tile_pool(name="mids", bufs=2))
        small = ctx.enter_context(tc.tile_pool(name="small", bufs=2))
        ups = ctx.enter_context(tc.tile_pool(name="ups", bufs=6))
        outs = ctx.enter_context(tc.tile_pool(name="outs", bufs=4))
        yps = ctx.enter_context(tc.tile_pool(name="yps", bufs=4, space="PSUM"))
        hps = ctx.enter_context(tc.tile_pool(name="hps", bufs=2, space="PSUM"))
        ops = ctx.enter_context(tc.tile_pool(name="ops", bufs=2, space="PSUM"))

        loop_cm = tc.For_i(0, loop_iters, 1) if loop_iters else nullcontext()
        with loop_cm:
          for _rep in range(n_reps):
            if _rep > 0 and serialize_reps:
                with tc.tile_critical():
                    nc.all_engine_barrier()

            # ---- t~0: scratch init + ACT table warm (Ln/Exp set) ------------
            warm = ins.tile([128, 640], mdt, tag="warm", bufs=2)
            if mdt == f32r:
                nc.vector.memset(warm[:].bitcast(f32), 0.0)
            else:
                nc.vector.memset(warm[:].bitcast(mybir.dt.uint16), 0)
            eps = small.tile([128, 1], f32, tag="eps")
            nc.vector.memset(eps, BN_EPS)
            tw = small.tile([128, 1], f32, tag="tw")
            nc.scalar.activation(tw[:], eps[:], Act.Ln)
            nc.scalar.activation(tw[:], tw[:], Act.Exp)

            # ---- input loads: spread across SP/ACT HWDGE + DVE HWDGE +
            # Pool SWDGE (each dma_start holds its queue for the whole
            # transfer in the HW model, so few-but-wide DMAs on many queues).
            # Need-order: {x-h0, w1, cbb} gate layer 1, so each rides its
            # own queue; second wave brings x-h1 / w2 / idd / gb.
            xall = ins.tile([128, KT1, B], mdt, tag="xts", name="xts", bufs=2)
            w1all = ins.tile([128, NPAIR, KT1, 128], mdt, tag="w1t",
                             name="w1t", bufs=2)
            w2all = ins.tile([128, NPAIR, MT2, 128], mdt, tag="w2t",
                             name="w2t", bufs=2)
            cbbt = ins.tile([E, CBB_W], mdt, tag="cbbt")
            iddt = ins.tile([128, 128], f32r, tag="iddt")
            gbt = small.tile([128, 2], f32, tag="gbt")
            w1v = w1all.rearrange("i p k j -> i (p k j)")
            w2v = w2all.rearrange("i p m j -> i (p m j)")
            nc.scalar.dma_start(cbbt[:], cbb[:])
            nc.sync.dma_start(w1v[:], w1l[:])
            for k in range(KT1):
                qs = nc.sync if k % 2 == 0 else nc.scalar
                qs.dma_start(xall[:, k, 0:512], xT[:, k, 0:512])
            nc.scalar.dma_start(iddt[:], idd[:])
            nc.sync.dma_start(gbt[:], gb[:])
            for k in range(KT1):
                qs = nc.sync if k % 2 == 0 else nc.scalar
                qs.dma_start(xall[:, k, 512:1024], xT[:, k, 512:1024])
            nc.sync.dma_start(w2v[:], w2l[:])

            ct = cbbt[:, 0:B]
            b1t = cbbt[:, B : B + 128]
            selt = cbbt[:, B + 128 : CBB_W]
            gm = gbt[:, 0:1]
            bt = gbt[:, 1:2]
            if phase < 1:
                continue

            # ---- PE p-state ramp during the DMA wait ------------------------
            for _w in range(N_WARM):
                wps = ops.tile([128, 512], f32, tag="ops", name="wps")
                nc.tensor.matmul(
                    wps[:], warm[:, 512:640], warm[:, 0:512],
                    start=True, stop=True,
                )

            # ---- coef broadcast tiles via PE + Pool copies ------------------
            ctiles = []
            for p in range(NPAIR):
                t = cbts.tile([128, B], mdt, tag="cbt", name="cbt")
                ctiles.append(t)
                for bh in range(NBH):
                    bsl = slice(bh * 512, (bh + 1) * 512)
                    cp = ops.tile([128, 512], f32, tag="ops", name="cp")
                    nc.tensor.matmul(
                        cp[:], selt[:, p * 128 : (p + 1) * 128], ct[:, bsl],
                        start=True, stop=True,
                    )
                    # gpsimd cannot read PSUM; split drains across DVE/ACT
                    if (p * NBH + bh) % 2 == 0:
                        nc.vector.tensor_copy(t[:, bsl], cp[:])
                    else:
                        nc.scalar.copy(t[:, bsl], cp[:])
            cbt = ctiles

            # ---- layer 1 (k-outer) + blend + pair-sum -----------------------
            h_ps = []
            stats = small.tile([128, NBH, 6], f32, tag="stats")
            for bh in range(NBH):
                bsl = slice(bh * 512, (bh + 1) * 512)
                hp = hps.tile([128, 512], f32, tag="hps")
                nc.tensor.matmul(hp[:], b1t, ct[:, bsl], start=True, stop=False)
                yts = []
                for k in range(KT1):
                    for p in range(NPAIR):
                        if k == 0:
                            yts.append(yps.tile([128, 512], f32, tag="yps",
                                                name="yp"))
                        nc.tensor.matmul(
                            yts[p][:], w1all[:, p, k, :], xall[:, k, bsl],
                            start=(k == 0), stop=(k == KT1 - 1),
                        )
                for p in range(NPAIR):
                    bl = blend.tile([128, 512], f32r, tag="bl")
                    nc.vector.tensor_mul(bl[:], yts[p][:], cbt[p][:, bsl])
                    nc.tensor.matmul(
                        hp[:], iddt[:], bl[:], start=False, stop=(p == NPAIR - 1)
                    )
                nc.vector.bn_stats(out=stats[:, bh, :], in_=hp[:])
                h_ps.append(hp)

            if phase < 2:
                continue
            # ---- batch-norm scale/bias --------------------------------------
            # rstd = exp(-0.5*ln(var+eps)): stays on the Ln/Exp ACT table.
            # (dummy matmuls keep the PE p-state hot through the serial tail)
            mv = small.tile([128, 2], f32, tag="mv")
            nc.vector.bn_aggr(out=mv[:], in_=stats[:])
            for _w in range(N_FILL):
                wps = ops.tile([128, 512], f32, tag="ops", name="wps")
                nc.tensor.matmul(
                    wps[:], warm[:, 512:640], warm[:, 0:512],
                    start=True, stop=True,
                )
            lnv = small.tile([128, 1], f32, tag="lnv")
            nc.scalar.activation(lnv[:], mv[:, 1:2], Act.Ln, bias=eps[:])
            rstd = small.tile([128, 1], f32, tag="rstd")
            nc.scalar.activation(rstd[:], lnv[:], Act.Exp, scale=-0.5)
            ns = small.tile([128, 1], f32, tag="ns")
            nc.vector.tensor_mul(ns[:], rstd[:], gm)
            nb0 = small.tile([128, 1], f32, tag="nb0")
            nc.vector.tensor_mul(nb0[:], mv[:, 0:1], ns[:])
            nb = small.tile([128, 1], f32, tag="nb")
            nc.vector.tensor_sub(nb[:], bt, nb0[:])

            # ---- ELU+1 + blend + layer 2 ------------------------------------
            oi = 0
            for bh in range(NBH if phase >= 3 else 0):
                bsl = slice(bh * 512, (bh + 1) * 512)
                expd = mids.tile([128, 512], f32, tag="expd")
                nc.scalar.activation(
                    expd[:], h_ps[bh][:], Act.Exp, bias=nb[:], scale=ns[:]
                )
                rl = mids.tile([128, 512], f32, tag="rl")
                nc.scalar.activation(
                    rl[:], h_ps[bh][:], Act.Relu, bias=nb[:], scale=ns[:]
                )
                h1 = mids.tile([128, 512], mdt, tag="h1")
                nc.vector.scalar_tensor_tensor(
                    out=h1[:], in0=expd[:], scalar=1.0, in1=rl[:],
                    op0=Alu.min, op1=Alu.add,
                )
                us = []
                for p in range(NPAIR):
                    u = ups.tile([128, 512], mdt, tag="u", name="u")
                    ueng = nc.vector if p % 2 == 0 else nc.gpsimd
                    ueng.tensor_mul(u[:], cbt[p][:, bsl], h1[:])
                    us.append(u)
                for m in range(MT2):
                    op = ops.tile([128, 512], f32, tag="ops")
                    for p in range(NPAIR):
                        nc.tensor.matmul(
                            op[:], w2all[:, p, m, :], us[p][:],
                            start=(p == 0), stop=(p == NPAIR - 1),
                        )
                    ot = outs.tile([128, 512], odt, tag="ot", name="ot")
                    if oi % 2 == 0:
                        nc.vector.tensor_copy(ot[:], op[:])
                    else:
                        nc.scalar.copy(ot[:], op[:])
                    seng = [nc.gpsimd, nc.sync, nc.scalar, nc.gpsimd,
                            nc.gpsimd, nc.sync, nc.scalar, nc.sync][oi % 8]
                    seng.dma_start(outT[m * 128 : (m + 1) * 128, bsl], ot[:])
                    oi += 1

    nc.compile()
    return nc


def make_in_maps(x, blending_coef, w1, b1, w2, b2, gamma, beta):
    """Host-side input marshaling: per-core weight slices + shared tensors."""
    global _HOST_BIAS_T
    import ml_dtypes

    f32 = np.float32
    mmdt = ml_dtypes.bfloat16 if IN_BF16 else f32
    x = np.asarray(x, f32)
    coef = np.asarray(blending_coef, f32)
    w1 = np.asarray(w1, f32)
    b1 = np.asarray(b1, f32)
    w2 = np.asarray(w2, f32)
    b2 = np.asarray(b2, f32)
    gamma = np.asarray(gamma, f32)
    beta = np.asarray(beta, f32)

    # blended bias for layer 2, including the correction for computing
    # (elu+1) on device:  out_true = out_dev + coef @ (b2 - sum_h w2[:,h,:])
    _HOST_BIAS_T = (
        coef.astype(np.float64) @ (b2 - w2.sum(axis=1)).astype(np.float64)
    ).T

    xT = np.ascontiguousarray(
        x.T.reshape(KT1, 128, B).transpose(1, 0, 2)
    )  # [128, KT1, B]
    coefT = np.ascontiguousarray(coef.T)
    idd = np.ascontiguousarray(np.tile(np.eye(64, dtype=f32), (2, 2)))
    sel = np.zeros((E, SELW), f32)
    for p in range(NPAIR):
        for q in range(2):
            sel[2 * p + q, p * 128 + q * 64 : p * 128 + (q + 1) * 64] = 1.0

    in_maps = []
    for c in range(N_CORES):
        sl = slice(c * HSL, (c + 1) * HSL)
        w1s = w1[:, :, sl]  # [E, IN, 64]
        w1L = np.ascontiguousarray(
            w1s.reshape(NPAIR, 2, KT1, 128, HSL)
            .transpose(3, 0, 2, 1, 4)  # [i, pair, k, eo, j]
            .reshape(128, NPAIR * KT1 * 128)
        )
        w2s = w2[:, sl, :]  # [E, 64, OUT]
        w2L = np.ascontiguousarray(
            w2s.reshape(NPAIR, 2, HSL, MT2, 128)
            .transpose(1, 2, 0, 3, 4)  # [eo, h, pair, m, j]
            .reshape(128, NPAIR * MT2 * 128)
        )
        cbbv = np.concatenate([coefT, np.tile(b1[:, sl], (1, 2)), sel], axis=1)
        gbv = np.stack(
            [np.tile(gamma[sl], 2), np.tile(beta[sl], 2)], axis=1
        )
        in_maps.append(
            {
                "xT": xT.astype(mmdt),
                "w1l": w1L.astype(mmdt),
                "w2l": w2L.astype(mmdt),
                "cbb": np.ascontiguousarray(cbbv).astype(mmdt),
                "gb": np.ascontiguousarray(gbv),
                "idd": idd,
            }
        )
    return in_maps


def combine_outputs(per_core_outs):
    """Sum per-core partial [OUT, B] outputs + host bias; return [B, OUT]."""
    acc = np.zeros((OUT, B), np.float64)
    for o in per_core_outs:
        acc += np.asarray(o, np.float64)
    if _HOST_BIAS_T is not None:
        acc += _HOST_BIAS_T
    return np.ascontiguousarray(acc.T.astype(np.float32))


def kernel(x, blending_coef, w1, b1, w2, b2, gamma, beta):
    global _CACHED
    from concourse.bass_utils import run_bass_kernel_spmd

    if _CACHED is None:
        _CACHED = build_nc(n_reps=1)
    nc = _CACHED
    in_maps = make_in_maps(x, blending_coef, w1, b1, w2, b2, gamma, beta)
    res = run_bass_kernel_spmd(nc, in_maps, list(range(N_CORES)))
    return combine_outputs([res.results[c]["outT"] for c in range(N_CORES)])


# revision 54
# speedup vs baseline: 1.3712x; 1.1318x over previous
"""Bass/Trainium2 kernel for nn_ExpertMLP (soft-blended 8-expert MLP with
BatchNorm between the two layers).

Math (per sample b):
    h  = sum_e coef[b,e] * (x[b] @ w1[e])  + coef[b] @ b1
    hn = (h - mean_B(h)) * rsqrt(var_B(h) + eps) * gamma + beta
    h1 = elu(hn)
    out= sum_e coef[b,e] * (h1[b] @ w2[e]) + coef[b] @ b2

Sharding: HID (512) is split 64-per-core across 8 cores. Each core processes
the FULL batch for its HID slice, so the BatchNorm batch statistics are fully
local (no collective), and the per-expert weights are sharded (not
replicated). Layer 2 contracts only the local HID slice, so each core emits a
partial output [OUT, B]; the host sums the 8 partials, adds the blended-bias
term coef @ (b2 - w2.sum(hid)) (the -w2sum corrects for computing elu+1 on
device), and transposes.

On-chip layout is transposed (features on partitions, batch on the free dim):
  - coef broadcast tiles cbt[p][q*64+r, b] = coef[b, 2p+q] are built on the
    PE: cbt_p = SEL_p^T @ coefT (SEL packed into the cbb constant), copied
    PSUM->SBUF on the (otherwise idle) gpsimd/Pool engine. This replaces 16
    tiny row-DMAs + 4 stream_shuffles (~1.6us of queue time EACH).
  - L1 (k-outer): yp_pair[(e0|e1)*64hid, b] += W1L[p,k].T @ xT[k,b] streams
    matmuls as x chunks land; blend multiplies by cbt on DVE/Pool; a tiled
    identity [I;I|I;I] matmul sums the two expert halves of each pair into
    the h PSUM accumulator (and duplicates h to both partition halves).
  - BN: bn_stats/bn_aggr over the free (batch) dim; rstd = exp(-0.5*ln(v+eps))
    so the ONLY ACT table used the whole kernel is natural_log_exp_and_others
    (Ln/Exp/Relu/Copy) -- no 1.3us mid-kernel table reloads. Dummy warm
    matmuls bridge the serial BN tail so the PE p-state stays at 2.4 GHz,
    and warmup matmuls at t=0 ramp the clock during the initial DMA wait.
  - ELU+1: relu(hn) + min(exp(hn), 1)  (the -1 is folded into the host bias).
  - L2: u_pair = cbt_p (*) [h1;h1]; out_m += W2L[p,m].T @ u_pair.
"""

import sys

sys.path.insert(0, "/opt/trn_rl_repo")

import numpy as np

E, IN, HID, OUT, B = 8, 512, 512, 512, 1024
BN_EPS = 1e-5
N_CORES = 8
HSL = HID // N_CORES  # 64: per-core hid slice
NPAIR = E // 2  # 4 expert pairs
KT1 = IN // 128  # 4 contraction tiles for layer 1
MT2 = OUT // 128  # 4 output row-tiles for layer 2
NBH = B // 512  # 2 batch halves (PSUM free-dim limit)
SELW = NPAIR * 128  # selection-matrix block for the cbt broadcast matmuls
CBB_W = B + 128 + SELW  # packed coefT | b1dup | SEL widths
IN_BF16 = True  # x/w1/w2/cbb in bf16 (halves input DMA; ~0.5% extra err)
OUT_BF16 = True  # partial outputs in bf16 (halves output DMA)
N_WARM = 6  # PE p-state warmup matmuls at t~0
N_FILL = 6  # PE keep-warm matmuls across the serial BN tail
USE_GPSIMD_U = False  # gpsimd tensor ops are slow on HW (software DSP)
USE_GPSIMD_OUT = True  # issue some output DMAs on the gpsimd SWDGE queue
DMA_PLAN = 2  # 0: x chunked on SP/ACT; 1: ACT DMA-light; 2: dedicated queues
OUT_COPY_ACT = True  # drain L2 PSUM via ACT only (keeps DVE for blends/u)
CP_ON_YPS = False  # cbt-gen PSUM tiles share the yps ring
L1_POUTER = True  # pair-major layer-1 sweep (blends/folds overlap mm stream)
L2_EARLY_ELU = True  # ELU+u for both batch halves before the L2 m-loops
YPS_BUFS = 4  # PSUM banks for the L1 pair accumulators (ops gets 8-2-this)
SPLIT_BH0 = True  # column-split bh0's ELU/u chain so L2 starts ~2x sooner
SPLIT_NBH = 1  # how many batch halves get the column-split chain (1 or 2)
CBT_COPY_ACT = False  # drain all cbt-gen PSUM via ACT (else alternate DVE/ACT)
OUT_WIDE = False  # one [128,1024] output DMA per m-tile (both batch halves)

_CACHED = None
_HOST_BIAS_T = None  # [OUT, B] float64, set by make_in_maps


def _l2_mloop(nc, ops, outs, w2all, us, outT, bh, oi, odt, f32, wide_ots=None):
    """Layer-2 m-tile loop for one batch half: accumulating matmuls per
    output row-tile (per column-chunk of u), PSUM drained via ACT copy,
    DMA'd on the out queues. us[p] is a list of column-chunk tiles.
    With wide_ots, both halves copy into one [128,1024] tile per m and a
    single DMA fires after the second half's copy."""
    bsl = slice(bh * 512, (bh + 1) * 512)
    nch = len(us[0])
    cw = 512 // nch
    for m in range(MT2):
        op = ops.tile([128, 512], f32, tag="ops", name="op")
        for h in range(nch):
            for p in range(NPAIR):
                nc.tensor.matmul(
                    op[:, h * cw : (h + 1) * cw], w2all[:, p, m, :],
                    us[p][h][:],
                    start=(p == 0), stop=(p == NPAIR - 1),
                )
        if wide_ots is not None:
            if bh == 0:
                wide_ots.append(outs.tile([128, 1024], odt, tag="otw",
                                          name="otw"))
            ot = wide_ots[m]
            if OUT_COPY_ACT or oi % 2 == 1:
                nc.scalar.copy(ot[:, bsl], op[:])
            else:
                nc.vector.tensor_copy(ot[:, bsl], op[:])
            if bh == NBH - 1:
                seng = nc.gpsimd if DMA_PLAN == 2 else \
                    [nc.gpsimd, nc.sync, nc.scalar, nc.gpsimd][m % 4]
                seng.dma_start(outT[m * 128 : (m + 1) * 128, :], ot[:])
            oi += 1
            continue
        ot = outs.tile([128, 512], odt, tag="ot", name="ot")
        if OUT_COPY_ACT or oi % 2 == 1:
            nc.scalar.copy(ot[:], op[:])
        else:
            nc.vector.tensor_copy(ot[:], op[:])
        if DMA_PLAN == 2:
            seng = nc.gpsimd
        elif USE_GPSIMD_OUT:
            seng = [nc.gpsimd, nc.sync, nc.scalar, nc.gpsimd,
                    nc.gpsimd, nc.sync, nc.scalar, nc.sync][oi % 8]
        else:
            seng = [nc.sync, nc.scalar][oi % 2]
        seng.dma_start(outT[m * 128 : (m + 1) * 128, bsl], ot[:])
        oi += 1
    return oi


def build_nc(n_reps: int = 1, trace_sim: bool = False, serialize_reps: bool = True,
             loop_iters: int = 0, phase: int = 3):
    """Build + compile the (SPMD, identical-program) Bass module."""
    from contextlib import ExitStack, nullcontext

    import concourse.bass as bass
    import concourse.tile as tile
    from concourse import bacc, mybir

    f32 = mybir.dt.float32
    f32r = mybir.dt.float32r
    bf16 = mybir.dt.bfloat16
    mdt = bf16 if IN_BF16 else f32r
    odt = bf16 if OUT_BF16 else f32
    Alu = mybir.AluOpType
    Act = mybir.ActivationFunctionType

    nc = bacc.Bacc(
        "TRN2", target_bir_lowering=False, debug=False, num_devices=N_CORES
    )

    xT = nc.dram_tensor("xT", [128, KT1, B], mdt, kind="ExternalInput")
    w1l = nc.dram_tensor("w1l", [128, NPAIR * KT1 * 128], mdt, kind="ExternalInput")
    w2l = nc.dram_tensor("w2l", [128, NPAIR * MT2 * 128], mdt, kind="ExternalInput")
    cbb = nc.dram_tensor("cbb", [E, CBB_W], mdt, kind="ExternalInput")
    gb = nc.dram_tensor("gb", [128, 2], f32, kind="ExternalInput")
    idd = nc.dram_tensor("idd", [128, 128], f32r, kind="ExternalInput")
    outT = nc.dram_tensor("outT", [OUT, B], odt, kind="ExternalOutput")

    with ExitStack() as ctx:
        tc = ctx.enter_context(tile.TileContext(nc, trace_sim=trace_sim))
        ins = ctx.enter_context(tc.tile_pool(name="ins", bufs=2))
        cbts = ctx.enter_context(tc.tile_pool(name="cbts", bufs=6))
        blend = ctx.enter_context(tc.tile_pool(name="blend", bufs=6))
        mids = ctx.enter_context(tc.tile_pool(name="mids", bufs=2))
        small = ctx.enter_context(tc.tile_pool(name="small", bufs=2))
        ups = ctx.enter_context(tc.tile_pool(name="ups", bufs=10))
        outs = ctx.enter_context(tc.tile_pool(name="outs", bufs=4))
        yps = ctx.enter_context(tc.tile_pool(name="yps", bufs=YPS_BUFS,
                                             space="PSUM"))
        hps = ctx.enter_context(tc.tile_pool(name="hps", bufs=2, space="PSUM"))
        ops = ctx.enter_context(tc.tile_pool(name="ops", bufs=8 - 2 - YPS_BUFS,
                                             space="PSUM"))

        loop_cm = tc.For_i(0, loop_iters, 1) if loop_iters else nullcontext()
        with loop_cm:
          for _rep in range(n_reps):
            if _rep > 0 and serialize_reps:
                with tc.tile_critical():
                    nc.all_engine_barrier()

            # ---- t~0: scratch init + ACT table warm (Ln/Exp set) ------------
            if N_WARM or N_FILL:
                warm = ins.tile([128, 640], mdt, tag="warm", bufs=2)
                if mdt == f32r:
                    nc.vector.memset(warm[:].bitcast(f32), 0.0)
                else:
                    nc.vector.memset(warm[:].bitcast(mybir.dt.uint16), 0)
            eps = small.tile([128, 1], f32, tag="eps")
            nc.vector.memset(eps, BN_EPS)
            tw = small.tile([128, 1], f32, tag="tw")
            nc.scalar.activation(tw[:], eps[:], Act.Ln)
            nc.scalar.activation(tw[:], tw[:], Act.Exp)
            if phase < 0:
                continue

            # ---- input loads: spread across SP/ACT HWDGE + DVE HWDGE +
            # Pool SWDGE (each dma_start holds its queue for the whole
            # transfer in the HW model, so few-but-wide DMAs on many queues).
            # Need-order: {x-h0, w1, cbb} gate layer 1, so each rides its
            # own queue; second wave brings x-h1 / w2 / idd / gb.
            xall = ins.tile([128, KT1, B], mdt, tag="xts", name="xts", bufs=2)
            w1all = ins.tile([128, NPAIR, KT1, 128], mdt, tag="w1t",
                             name="w1t", bufs=2)
            w2all = ins.tile([128, NPAIR, MT2, 128], mdt, tag="w2t",
                             name="w2t", bufs=2)
            cbbt = ins.tile([E, CBB_W], mdt, tag="cbbt")
            iddt = ins.tile([128, 128], f32r, tag="iddt")
            gbt = small.tile([128, 2], f32, tag="gbt")
            w1v = w1all.rearrange("i p k j -> i (p k j)")
            w2v = w2all.rearrange("i p m j -> i (p m j)")
            if DMA_PLAN == 2:
                # Dedicated queues: SP = inputs only (prefetches a full
                # iteration ahead since no compute-gated DMAs block it);
                # Pool/SWDGE = idd+w2+all outputs; ACT = compute only.
                nc.sync.dma_start(cbbt[:], cbb[:])
                nc.sync.dma_start(xall[:, :, 0:512], xT[:, :, 0:512])
                nc.sync.dma_start(w1v[:], w1l[:])
                nc.sync.dma_start(xall[:, :, 512:1024], xT[:, :, 512:1024])
                nc.sync.dma_start(gbt[:], gb[:])
                nc.gpsimd.dma_start(iddt[:], idd[:])
                nc.gpsimd.dma_start(w2v[:], w2l[:])
            elif DMA_PLAN == 1:
                # ACT issues only w1 (early, before its compute); x rides SP;
                # small/contiguous tensors go via the gpsimd SWDGE queue.
                nc.gpsimd.dma_start(cbbt[:], cbb[:])
                nc.scalar.dma_start(w1v[:], w1l[:])
                nc.sync.dma_start(xall[:, :, 0:512], xT[:, :, 0:512])
                nc.gpsimd.dma_start(iddt[:], idd[:])
                nc.sync.dma_start(xall[:, :, 512:1024], xT[:, :, 512:1024])
                nc.gpsimd.dma_start(w2v[:], w2l[:])
                nc.sync.dma_start(gbt[:], gb[:])
            else:
                nc.scalar.dma_start(cbbt[:], cbb[:])
                nc.sync.dma_start(w1v[:], w1l[:])
                for k in range(KT1):
                    qs = nc.sync if k % 2 == 0 else nc.scalar
                    qs.dma_start(xall[:, k, 0:512], xT[:, k, 0:512])
                nc.scalar.dma_start(iddt[:], idd[:])
                nc.sync.dma_start(gbt[:], gb[:])
                for k in range(KT1):
                    qs = nc.sync if k % 2 == 0 else nc.scalar
                    qs.dma_start(xall[:, k, 512:1024], xT[:, k, 512:1024])
                nc.sync.dma_start(w2v[:], w2l[:])

            ct = cbbt[:, 0:B]
            b1t = cbbt[:, B : B + 128]
            selt = cbbt[:, B + 128 : CBB_W]
            gm = gbt[:, 0:1]
            bt = gbt[:, 1:2]
            if phase < 1:
                continue

            # ---- PE p-state ramp during the DMA wait ------------------------
            for _w in range(N_WARM):
                wps = ops.tile([128, 512], f32, tag="ops", name="wps")
                nc.tensor.matmul(
                    wps[:], warm[:, 512:640], warm[:, 0:512],
                    start=True, stop=True,
                )
            cpool, ctag = (yps, "yps") if CP_ON_YPS else (ops, "ops")

            # ---- coef broadcast tiles via PE + Pool copies ------------------
            ctiles = []
            for p in range(NPAIR):
                t = cbts.tile([128, B], mdt, tag="cbt", name="cbt")
                ctiles.append(t)
                for bh in range(NBH):
                    bsl = slice(bh * 512, (bh + 1) * 512)
                    cp = cpool.tile([128, 512], f32, tag=ctag, name="cp")
                    nc.tensor.matmul(
                        cp[:], selt[:, p * 128 : (p + 1) * 128], ct[:, bsl],
                        start=True, stop=True,
                    )
                    # gpsimd cannot read PSUM; split drains across DVE/ACT
                    if CBT_COPY_ACT or (p * NBH + bh) % 2 == 1:
                        nc.scalar.copy(t[:, bsl], cp[:])
                    else:
                        nc.vector.tensor_copy(t[:, bsl], cp[:])
            cbt = ctiles

            # ---- layer 1 (k-outer) + blend + pair-sum -----------------------
            h_ps = []
            stats = small.tile([128, NBH, 6], f32, tag="stats")
            for bh in range(NBH):
                bsl = slice(bh * 512, (bh + 1) * 512)
                hp = hps.tile([128, 512], f32, tag="hps")
                nc.tensor.matmul(hp[:], b1t, ct[:, bsl], start=True, stop=False)
                yts = []
                if L1_POUTER:
                    # pair-major: each pair's accumulation closes early so its
                    # blend (DVE) and fold (PE) overlap the later pairs' mms
                    bls = []
                    for p in range(NPAIR):
                        yp = yps.tile([128, 512], f32, tag="yps", name="yp")
                        for k in range(KT1):
                            nc.tensor.matmul(
                                yp[:], w1all[:, p, k, :], xall[:, k, bsl],
                                start=(k == 0), stop=(k == KT1 - 1),
                            )
                        bl = blend.tile([128, 512], f32r, tag="bl")
                        nc.vector.tensor_mul(bl[:], yp[:], cbt[p][:, bsl])
                        bls.append(bl)
                    for p in range(NPAIR):
                        nc.tensor.matmul(
                            hp[:], iddt[:], bls[p][:],
                            start=False, stop=(p == NPAIR - 1),
                        )
                else:
                    for k in range(KT1):
                        for p in range(NPAIR):
                            if k == 0:
                                yts.append(yps.tile([128, 512], f32, tag="yps",
                                                    name="yp"))
                            nc.tensor.matmul(
                                yts[p][:], w1all[:, p, k, :], xall[:, k, bsl],
                                start=(k == 0), stop=(k == KT1 - 1),
                            )
                    for p in range(NPAIR):
                        bl = blend.tile([128, 512], f32r, tag="bl")
                        nc.vector.tensor_mul(bl[:], yts[p][:], cbt[p][:, bsl])
                        nc.tensor.matmul(
                            hp[:], iddt[:], bl[:], start=False, stop=(p == NPAIR - 1)
                        )
                nc.vector.bn_stats(out=stats[:, bh, :], in_=hp[:])
                h_ps.append(hp)

            if phase < 2:
                continue
            # ---- batch-norm scale/bias --------------------------------------
            # rstd = exp(-0.5*ln(var+eps)): stays on the Ln/Exp ACT table.
            # (dummy matmuls keep the PE p-state hot through the serial tail)
            mv = small.tile([128, 2], f32, tag="mv")
            nc.vector.bn_aggr(out=mv[:], in_=stats[:])
            for _w in range(N_FILL):
                wps = ops.tile([128, 512], f32, tag="ops", name="wps")
                nc.tensor.matmul(
                    wps[:], warm[:, 512:640], warm[:, 0:512],
                    start=True, stop=True,
                )
            lnv = small.tile([128, 1], f32, tag="lnv")
            nc.scalar.activation(lnv[:], mv[:, 1:2], Act.Ln, bias=eps[:])
            rstd = small.tile([128, 1], f32, tag="rstd")
            nc.scalar.activation(rstd[:], lnv[:], Act.Exp, scale=-0.5)
            ns = small.tile([128, 1], f32, tag="ns")
            nc.vector.tensor_mul(ns[:], rstd[:], gm)
            nb0 = small.tile([128, 1], f32, tag="nb0")
            nc.vector.tensor_mul(nb0[:], mv[:, 0:1], ns[:])
            nb = small.tile([128, 1], f32, tag="nb")
            nc.vector.tensor_sub(nb[:], bt, nb0[:])

            # ---- ELU+1 + blend + layer 2 ------------------------------------
            # ELU+u for BOTH batch halves are emitted ahead of the m-loops so
            # bh1's u tiles are produced (DVE/ACT) while the PE runs bh0's L2.
            oi = 0
            all_us = []
            for bh in range(NBH if phase >= 3 else 0):
                bsl = slice(bh * 512, (bh + 1) * 512)
                # bh0 gates the PE after the BN tail: split its chain into
                # two 256-col halves so the first L2 matmuls start sooner.
                nch = 2 if (SPLIT_BH0 and bh < SPLIT_NBH) else 1
                cw = 512 // nch
                us = [[] for _ in range(NPAIR)]
                for h in range(nch):
                    hsl = slice(bh * 512 + h * cw, bh * 512 + (h + 1) * cw)
                    psl = slice(h * cw, (h + 1) * cw)
                    expd = mids.tile([128, cw], f32, tag=f"expd{nch}{h}")
                    nc.scalar.activation(
                        expd[:], h_ps[bh][:, psl], Act.Exp,
                        bias=nb[:], scale=ns[:]
                    )
                    rl = mids.tile([128, cw], f32, tag=f"rl{nch}{h}")
                    nc.scalar.activation(
                        rl[:], h_ps[bh][:, psl], Act.Relu,
                        bias=nb[:], scale=ns[:]
                    )
                    h1 = mids.tile([128, cw], mdt, tag=f"h1{nch}{h}")
                    nc.vector.scalar_tensor_tensor(
                        out=h1[:], in0=expd[:], scalar=1.0, in1=rl[:],
                        op0=Alu.min, op1=Alu.add,
                    )
                    for p in range(NPAIR):
                        u = ups.tile([128, cw], mdt, tag=f"u{nch}{h}", name="u")
                        ueng = (nc.gpsimd if (USE_GPSIMD_U and p % 2 == 1)
                                else nc.vector)
                        ueng.tensor_mul(u[:], cbt[p][:, hsl], h1[:])
                        us[p].append(u)
                all_us.append(us)
                if not L2_EARLY_ELU:
                    oi = _l2_mloop(nc, ops, outs, w2all, us, outT, bh, oi,
                                   odt, f32)
            if L2_EARLY_ELU:
                wide_ots = [] if OUT_WIDE else None
                for bh in range(NBH if phase >= 3 else 0):
                    oi = _l2_mloop(nc, ops, outs, w2all, all_us[bh], outT, bh,
                                   oi, odt, f32, wide_ots=wide_ots)

    nc.compile()
    return nc


def make_in_maps(x, blending_coef, w1, b1, w2, b2, gamma, beta):
    """Host-side input marshaling: per-core weight slices + shared tensors."""
    global _HOST_BIAS_T
    import ml_dtypes

    f32 = np.float32
    mmdt = ml_dtypes.bfloat16 if IN_BF16 else f32
    x = np.asarray(x, f32)
    coef = np.asarray(blending_coef, f32)
    w1 = np.asarray(w1, f32)
    b1 = np.asarray(b1, f32)
    w2 = np.asarray(w2, f32)
    b2 = np.asarray(b2, f32)
    gamma = np.asarray(gamma, f32)
    beta = np.asarray(beta, f32)

    # blended bias for layer 2, including the correction for computing
    # (elu+1) on device:  out_true = out_dev + coef @ (b2 - sum_h w2[:,h,:])
    _HOST_BIAS_T = (
        coef.astype(np.float64) @ (b2 - w2.sum(axis=1)).astype(np.float64)
    ).T

    xT = np.ascontiguousarray(
        x.T.reshape(KT1, 128, B).transpose(1, 0, 2)
    )  # [128, KT1, B]
    coefT = np.ascontiguousarray(coef.T)
    idd = np.ascontiguousarray(np.tile(np.eye(64, dtype=f32), (2, 2)))
    sel = np.zeros((E, SELW), f32)
    for p in range(NPAIR):
        for q in range(2):
            sel[2 * p + q, p * 128 + q * 64 : p * 128 + (q + 1) * 64] = 1.0

    in_maps = []
    for c in range(N_CORES):
        sl = slice(c * HSL, (c + 1) * HSL)
        w1s = w1[:, :, sl]  # [E, IN, 64]
        w1L = np.ascontiguousarray(
            w1s.reshape(NPAIR, 2, KT1, 128, HSL)
            .transpose(3, 0, 2, 1, 4)  # [i, pair, k, eo, j]
            .reshape(128, NPAIR * KT1 * 128)
        )
        w2s = w2[:, sl, :]  # [E, 64, OUT]
        w2L = np.ascontiguousarray(
            w2s.reshape(NPAIR, 2, HSL, MT2, 128)
            .transpose(1, 2, 0, 3, 4)  # [eo, h, pair, m, j]
            .reshape(128, NPAIR * MT2 * 128)
        )
        cbbv = np.concatenate([coefT, np.tile(b1[:, sl], (1, 2)), sel], axis=1)
        gbv = np.stack(
            [np.tile(gamma[sl], 2), np.tile(beta[sl], 2)], axis=1
        )
        in_maps.append(
            {
                "xT": xT.astype(mmdt),
                "w1l": w1L.astype(mmdt),
                "w2l": w2L.astype(mmdt),
                "cbb": np.ascontiguousarray(cbbv).astype(mmdt),
                "gb": np.ascontiguousarray(gbv),
                "idd": idd,
            }
        )
    return in_maps


def combine_outputs(per_core_outs):
    """Sum per-core partial [OUT, B] outputs + host bias; return [B, OUT]."""
    acc = np.zeros((OUT, B), np.float64)
    for o in per_core_outs:
        acc += np.asarray(o, np.float64)
    if _HOST_BIAS_T is not None:
        acc += _HOST_BIAS_T
    return np.ascontiguousarray(acc.T.astype(np.float32))


def kernel(x, blending_coef, w1, b1, w2, b2, gamma, beta):
    global _CACHED
    from concourse.bass_utils import run_bass_kernel_spmd

    if _CACHED is None:
        _CACHED = build_nc(n_reps=1)
    nc = _CACHED
    in_maps = make_in_maps(x, blending_coef, w1, b1, w2, b2, gamma, beta)
    res = run_bass_kernel_spmd(nc, in_maps, list(range(N_CORES)))
    return combine_outputs([res.results[c]["outT"] for c in range(N_CORES)])
